# revision 1
# baseline (speedup 1.0000x reference)
"""DKT next-question BCE loss on 8 trn2 NeuronCores.

Data-parallel over the student axis: 32 students per core. Host-side
prep aligns pred[t] with batch[t+1] (the loss pairs step t's prediction
with step t+1's attempted question), flattens (student, step) into rows
and zero-pads to 6400 rows per core. On device, each 128-row group does
two fused multiply-reduce ops (scalar_tensor_tensor + accum_out) on the
vector engine:
  s1[r]  =  sum_q pred[r,q] * batch[r, q]       (correct-answer half)
  s2n[r] = -sum_q pred[r,q] * batch[r, Q+q]     (incorrect-answer half)
Because batch rows are one-hot * correctness, v = s1 + s2n is +prob if
the row was answered correctly, -prob if not, and 0 for padded/empty
rows — so p = |v|, a = [v>0], mask = [v!=0]. The BCE tail runs
per-iteration on tiny [128,G] stats so it overlaps the DMA stream; the
iteration schedule tapers (9x G=5, then 5x G=1) so almost no compute is
exposed after the last DMA. Per-partition partials return to the host,
which sums across partitions and cores (the all-reduce of the scalar
loss) and negates.
"""

import sys

import numpy as np

sys.path.insert(0, "/opt/trn_rl_repo")

import concourse.bacc as bacc
import concourse.mybir as mybir
import concourse.tile as tile
from concourse.bass_utils import run_bass_kernel_spmd

B, T, Q = 256, 200, 1024
NCORES = 8
BS = B // NCORES              # students per core
ROWS = BS * (T - 1)           # 6368 valid rows per core
RPAD = 6400                   # padded rows
# Each partition covers 2 adjacent DRAM rows (8KB/16KB descriptors);
# one "group" = 256 rows. Schedule tapers so the final iterations leave
# almost no compute exposed after the last DMA.
SCHEDULE = [2] * 10 + [1] * 5  # 256-row groups per iteration (sum = 25)
NITER = len(SCHEDULE)

F32 = mybir.dt.float32
_cache: dict = {}


def _build():
    nc = bacc.Bacc("TRN2", target_bir_lowering=False, debug=False,
                   num_devices=NCORES)
    pred_h = nc.dram_tensor("pred", [RPAD, Q], F32, kind="ExternalInput")
    batch_h = nc.dram_tensor("batch", [RPAD, 2 * Q], F32, kind="ExternalInput")
    out_h = nc.dram_tensor("out", [128, 1], F32, kind="ExternalOutput")

    mult = mybir.AluOpType.mult
    add = mybir.AluOpType.add
    Ln = mybir.ActivationFunctionType.Ln
    Abs = mybir.ActivationFunctionType.Abs

    with tile.TileContext(nc) as tc:
        with tc.tile_pool(name="pred_p", bufs=3) as pp, \
             tc.tile_pool(name="batch_p", bufs=3) as bp, \
             tc.tile_pool(name="prod_p", bufs=2) as sp, \
             tc.tile_pool(name="tail_p", bufs=2) as tp, \
             tc.tile_pool(name="acc_p", bufs=1) as ac:
            lsum = ac.tile([128, NITER], F32)
            off = 0
            for i, G in enumerate(SCHEDULE):
                NC_ = 2 * G  # stat columns this iteration (one per row)
                pt = pp.tile([128, G, 2, Q], F32, tag="pt")
                bt = bp.tile([128, G, 2, 2 * Q], F32, tag="bt")
                rows = slice(off, off + G * 256)
                off += G * 256
                # both input streams issue from sync, which runs no
                # compute — DMA prefetch never waits on the compute
                # pipeline (scalar carries the BCE activations)
                nc.sync.dma_start(
                    out=pt[:],
                    in_=pred_h[rows, :].rearrange("(g p h) q -> p g h q",
                                                  p=128, h=2))
                nc.sync.dma_start(
                    out=bt[:],
                    in_=batch_h[rows, :].rearrange("(g p h) q -> p g h q",
                                                   p=128, h=2))
                s1 = tp.tile([128, NC_], F32, tag="s1")
                s2n = tp.tile([128, NC_], F32, tag="s2n")
                for g in range(G):
                    for h in range(2):
                        k = 2 * g + h
                        prod = sp.tile([128, Q], F32, tag="prod")
                        nc.vector.scalar_tensor_tensor(
                            out=prod[:], in0=pt[:, g, h, :], scalar=1.0,
                            in1=bt[:, g, h, 0:Q], op0=mult, op1=mult,
                            accum_out=s1[:, k:k + 1])
                        prod2 = sp.tile([128, Q], F32, tag="prod")
                        nc.vector.scalar_tensor_tensor(
                            out=prod2[:], in0=pt[:, g, h, :], scalar=-1.0,
                            in1=bt[:, g, h, Q:2 * Q], op0=mult, op1=mult,
                            accum_out=s2n[:, k:k + 1])

                # BCE tail for this iteration's columns, overlapped
                # with the next iterations' DMA.
                G = NC_
                v = tp.tile([128, G], F32, tag="v")
                nc.vector.tensor_add(v[:], s1[:], s2n[:])
                p = tp.tile([128, G], F32, tag="p")
                nc.scalar.activation(p[:], v[:], Abs)
                a = tp.tile([128, G], F32, tag="a")
                nc.vector.tensor_scalar(out=a[:], in0=v[:], scalar1=0.0,
                                        scalar2=None,
                                        op0=mybir.AluOpType.is_gt)
                mask = tp.tile([128, G], F32, tag="mask")
                nc.vector.tensor_scalar(out=mask[:], in0=v[:], scalar1=0.0,
                                        scalar2=None,
                                        op0=mybir.AluOpType.not_equal)
                # safe p: 0.5 where v == 0 so Ln stays finite
                eq = tp.tile([128, G], F32, tag="eq")
                nc.vector.tensor_scalar(out=eq[:], in0=v[:], scalar1=0.0,
                                        scalar2=None,
                                        op0=mybir.AluOpType.is_equal)
                half = tp.tile([128, G], F32, tag="half")
                nc.vector.tensor_scalar(out=half[:], in0=eq[:], scalar1=0.5,
                                        scalar2=None, op0=mult)
                spf = tp.tile([128, G], F32, tag="spf")
                nc.vector.tensor_add(spf[:], half[:], p[:])
                lp = tp.tile([128, G], F32, tag="lp")
                nc.scalar.activation(lp[:], spf[:], Ln)
                lq = tp.tile([128, G], F32, tag="lq")
                nc.scalar.activation(lq[:], spf[:], Ln, bias=1.0, scale=-1.0)
                # ll = a*lp + (1-a)*lq, then mask out empty rows
                d = tp.tile([128, G], F32, tag="d")
                nc.vector.tensor_sub(d[:], lp[:], lq[:])
                ad = tp.tile([128, G], F32, tag="ad")
                nc.vector.tensor_mul(ad[:], a[:], d[:])
                ll = tp.tile([128, G], F32, tag="ll")
                nc.vector.tensor_add(ll[:], lq[:], ad[:])
                llm = tp.tile([128, G], F32, tag="llm")
                nc.vector.tensor_mul(llm[:], ll[:], mask[:])
                nc.vector.tensor_reduce(out=lsum[:, i:i + 1], in_=llm[:],
                                        axis=mybir.AxisListType.X, op=add)

            part = ac.tile([128, 1], F32)
            nc.vector.tensor_reduce(out=part[:], in_=lsum[:],
                                    axis=mybir.AxisListType.X, op=add)
            nc.sync.dma_start(out=out_h[:], in_=part[:])

    nc.compile()
    return nc


def _get_nc():
    if "nc" not in _cache:
        _cache["nc"] = _build()
    return _cache["nc"]


def _in_maps(pred: np.ndarray, batch: np.ndarray) -> list[dict]:
    pred = np.asarray(pred, dtype=np.float32)
    batch = np.asarray(batch, dtype=np.float32)
    maps = []
    for c in range(NCORES):
        sl = slice(c * BS, (c + 1) * BS)
        pc = np.zeros((RPAD, Q), np.float32)
        pc[:ROWS] = pred[sl, :T - 1, :].reshape(ROWS, Q)
        bc = np.zeros((RPAD, 2 * Q), np.float32)
        bc[:ROWS] = batch[sl, 1:, :].reshape(ROWS, 2 * Q)
        maps.append({"pred": pc, "batch": bc})
    return maps


def _axon_reset():
    """Best-effort device reset: clears wedged NRT state on the terminal
    left by previously crashed runs. No-op if the axon .so is absent."""
    try:
        import ctypes

        import jax
        jax.devices()
        lib = ctypes.CDLL("/opt/axon/libaxon_pjrt.so")
        lib.axon_reset.restype = ctypes.c_int64
        lib.axon_reset()
    except Exception:
        pass


def _run(pred: np.ndarray, batch: np.ndarray, trace: bool = False,
         all_cores: bool = False):
    nc = _get_nc()
    _axon_reset()
    kw = {"trace_cores": list(range(NCORES))} if all_cores else {}
    res = run_bass_kernel_spmd(nc, _in_maps(pred, batch),
                               list(range(NCORES)), trace=trace, **kw)
    total = np.sum([np.asarray(r["out"], np.float64).sum()
                    for r in res.results])
    loss = np.array([-total], dtype=np.float32)
    return loss, res


def kernel(pred: np.ndarray, batch: np.ndarray) -> np.ndarray:
    loss, _ = _run(pred, batch)
    return loss



# revision 2
# speedup vs baseline: 1.2819x; 1.2819x over previous
"""DKT next-question BCE loss on 8 trn2 NeuronCores.

Data-parallel over the student axis: 32 students per core. Host-side
prep aligns pred[t] with batch[t+1] (the loss pairs step t's prediction
with step t+1's attempted question), flattens (student, step) into rows
and zero-pads to 6400 rows per core.

The kernel is HBM-bandwidth bound, so the inputs are shipped in compact
encodings: pred as fp16 (the 2e-2 loss tolerance leaves ~180x headroom
for fp16 rounding; validated against the f32 reference) and batch as
int8 (its values are exactly 0.0/1.0, so the cast is lossless). That's
26.2 MB per core instead of 78.6 MB. On device, each 128-row group does
two fused multiply-reduce ops (scalar_tensor_tensor, mixed fp16*int8)
on the vector engine:
  s1[r]  =  sum_q pred[r,q] * batch[r, q]       (correct-answer half)
  s2n[r] = -sum_q pred[r,q] * batch[r, Q+q]     (incorrect-answer half)
Because batch rows are one-hot * correctness, v = s1 + s2n is +prob if
the row was answered correctly, -prob if not, and 0 for padded rows —
so p = |v|, a = [v>0]. The per-row stats accumulate into one [128,50]
tile and the BCE tail runs ONCE at the end (9 wide ops) instead of 25
tiny per-iteration tails, removing ~20us of fixed instruction overhead
from the vector engine. p is clamped to [1e-6, 1-2^-11] so fp16 values
that rounded to exactly 1.0 keep log1p(-p) finite; padded rows
contribute log1p(-1e-6) ~ -1e-6, which is negligible vs the 2e-2
tolerance. Per-partition partials return to the host, which sums across
partitions and cores (the all-reduce of the scalar loss) and negates.
"""

import sys

import numpy as np

sys.path.insert(0, "/opt/trn_rl_repo")

import concourse.bacc as bacc
import concourse.mybir as mybir
import concourse.tile as tile
from concourse.bass_utils import run_bass_kernel_spmd

B, T, Q = 256, 200, 1024
NCORES = 8
BS = B // NCORES              # students per core
ROWS = BS * (T - 1)           # 6368 valid rows per core
RPAD = 6400                   # padded rows
# Each partition covers 2 adjacent DRAM rows (4KB descriptors in fp16/
# int8); one "group" = 256 rows. Schedule tapers so the final iterations
# leave almost no compute exposed after the last DMA.
SCHEDULE = [2] * 10 + [1] * 5  # 256-row groups per iteration (sum = 25)
NITER = len(SCHEDULE)
NCOLS = 2 * sum(SCHEDULE)      # one stat column per (partition, row pair)

CLAMP_HI = 1.0 - 2.0 ** -11    # largest fp16 < 1 (keeps log1p(-p) finite)
CLAMP_LO = 1e-6                # padded rows: ll = log1p(-1e-6) ~ 0

F32 = mybir.dt.float32
F16 = mybir.dt.float16
I8 = mybir.dt.int8
_cache: dict = {}


def _build():
    nc = bacc.Bacc("TRN2", target_bir_lowering=False, debug=False,
                   num_devices=NCORES)
    pred_h = nc.dram_tensor("pred", [RPAD, Q], F16, kind="ExternalInput")
    batch_h = nc.dram_tensor("batch", [RPAD, 2 * Q], I8, kind="ExternalInput")
    out_h = nc.dram_tensor("out", [128, 1], F32, kind="ExternalOutput")

    mult = mybir.AluOpType.mult
    add = mybir.AluOpType.add
    Ln = mybir.ActivationFunctionType.Ln
    Abs = mybir.ActivationFunctionType.Abs

    with tile.TileContext(nc) as tc:
        with tc.tile_pool(name="pred_p", bufs=3) as pp, \
             tc.tile_pool(name="batch_p", bufs=3) as bp, \
             tc.tile_pool(name="prod_p", bufs=2) as sp, \
             tc.tile_pool(name="tail_p", bufs=1) as tp, \
             tc.tile_pool(name="acc_p", bufs=1) as ac:
            s1 = ac.tile([128, NCOLS], F32)
            s2n = ac.tile([128, NCOLS], F32)
            off = 0
            col = 0
            for G in SCHEDULE:
                pt = pp.tile([128, G, 2, Q], F16, tag="pt")
                bt = bp.tile([128, G, 2, 2 * Q], I8, tag="bt")
                rows = slice(off, off + G * 256)
                off += G * 256
                # both input streams issue from sync, which runs no
                # compute — DMA prefetch never waits on the compute
                # pipeline
                nc.sync.dma_start(
                    out=pt[:],
                    in_=pred_h[rows, :].rearrange("(g p h) q -> p g h q",
                                                  p=128, h=2))
                nc.sync.dma_start(
                    out=bt[:],
                    in_=batch_h[rows, :].rearrange("(g p h) q -> p g h q",
                                                   p=128, h=2))
                for g in range(G):
                    for h in range(2):
                        prod = sp.tile([128, Q], F16, tag="prod")
                        nc.vector.scalar_tensor_tensor(
                            out=prod[:], in0=pt[:, g, h, :], scalar=1.0,
                            in1=bt[:, g, h, 0:Q], op0=mult, op1=mult,
                            accum_out=s1[:, col:col + 1])
                        prod2 = sp.tile([128, Q], F16, tag="prod")
                        nc.vector.scalar_tensor_tensor(
                            out=prod2[:], in0=pt[:, g, h, :], scalar=-1.0,
                            in1=bt[:, g, h, Q:2 * Q], op0=mult, op1=mult,
                            accum_out=s2n[:, col:col + 1])
                        col += 1

            # BCE tail, once, over all [128, NCOLS] stats.
            v = tp.tile([128, NCOLS], F32)
            nc.vector.tensor_add(v[:], s1[:], s2n[:])
            p = tp.tile([128, NCOLS], F32)
            nc.scalar.activation(p[:], v[:], Abs)
            spf = tp.tile([128, NCOLS], F32)
            nc.vector.tensor_scalar(out=spf[:], in0=p[:],
                                    scalar1=CLAMP_HI, scalar2=CLAMP_LO,
                                    op0=mybir.AluOpType.min,
                                    op1=mybir.AluOpType.max)
            a = tp.tile([128, NCOLS], F32)
            nc.vector.tensor_scalar(out=a[:], in0=v[:], scalar1=0.0,
                                    scalar2=None,
                                    op0=mybir.AluOpType.is_gt)
            lp = tp.tile([128, NCOLS], F32)
            nc.scalar.activation(lp[:], spf[:], Ln)
            lq = tp.tile([128, NCOLS], F32)
            nc.scalar.activation(lq[:], spf[:], Ln, bias=1.0, scale=-1.0)
            # ll = a*lp + (1-a)*lq = lq + a*(lp-lq)
            d = tp.tile([128, NCOLS], F32)
            nc.vector.tensor_sub(d[:], lp[:], lq[:])
            ad = tp.tile([128, NCOLS], F32)
            nc.vector.tensor_mul(ad[:], a[:], d[:])
            ll = tp.tile([128, NCOLS], F32)
            nc.vector.tensor_add(ll[:], lq[:], ad[:])
            part = ac.tile([128, 1], F32)
            nc.vector.tensor_reduce(out=part[:], in_=ll[:],
                                    axis=mybir.AxisListType.X, op=add)
            nc.sync.dma_start(out=out_h[:], in_=part[:])

    nc.compile()
    return nc


def _get_nc():
    if "nc" not in _cache:
        _cache["nc"] = _build()
    return _cache["nc"]


def _in_maps(pred: np.ndarray, batch: np.ndarray) -> list[dict]:
    pred = np.asarray(pred)
    batch = np.asarray(batch)
    maps = []
    for c in range(NCORES):
        sl = slice(c * BS, (c + 1) * BS)
        pc = np.zeros((RPAD, Q), np.float16)
        pc[:ROWS] = pred[sl, :T - 1, :].reshape(ROWS, Q).astype(np.float16)
        bc = np.zeros((RPAD, 2 * Q), np.int8)
        bc[:ROWS] = batch[sl, 1:, :].reshape(ROWS, 2 * Q).astype(np.int8)
        maps.append({"pred": pc, "batch": bc})
    return maps


def _axon_reset():
    """Best-effort device reset: clears wedged NRT state on the terminal
    left by previously crashed runs. No-op if the axon .so is absent."""
    try:
        import ctypes

        import jax
        jax.devices()
        lib = ctypes.CDLL("/opt/axon/libaxon_pjrt.so")
        lib.axon_reset.restype = ctypes.c_int64
        lib.axon_reset()
    except Exception:
        pass


def _run(pred: np.ndarray, batch: np.ndarray, trace: bool = False,
         all_cores: bool = False):
    nc = _get_nc()
    _axon_reset()
    kw = {"trace_cores": list(range(NCORES))} if all_cores else {}
    res = run_bass_kernel_spmd(nc, _in_maps(pred, batch),
                               list(range(NCORES)), trace=trace, **kw)
    total = np.sum([np.asarray(r["out"], np.float64).sum()
                    for r in res.results])
    loss = np.array([-total], dtype=np.float32)
    return loss, res


def kernel(pred: np.ndarray, batch: np.ndarray) -> np.ndarray:
    loss, _ = _run(pred, batch)
    return loss


# revision 10
# speedup vs baseline: 2.0520x; 1.6008x over previous
"""DKT next-question BCE loss on 8 trn2 NeuronCores.

Data-parallel over students (32/core, 6368 valid rows + pad to 6400).
The loss touches ONE pred element per row (the one-hot row-dot), so the
HBM traffic floor is what decides performance. Batch ships bit-packed
(256B/row -> 1.6MB/core, a lossless re-encode of its exact 0.0/1.0
one-hot values) and pred ships as fp16 (13MB/core; the 2e-2 loss
tolerance leaves ~180x headroom for fp16 rounding, validated against
the f32 reference). All decoding happens on device, in 5 pipelined
1280-row windows:

1. XBAR transpose-load: packed batch rows [1280, 128 uint16] -> SBUF
   [128 words, 1280 rows]; word c of a row holds one-hot bits for
   elements j = 16c+t.
2. The idle tensor engine finds the one-hot position: words convert to
   fp16 (values 2^t exactly; DVE 4x tensor_copy) and two matmul columns
   [1, (c+1)/128] contract over the 128 word-partitions: F1 = 2^t,
   F2 = (c+1)/128 * 2^t, landing each row on its own PSUM partition.
3. f32 bit tricks decode (c, t) exactly on [128, 10] stats: t from F1's
   exponent field, 1/F1 = bitcast(0x7F000000 - bits(F1)),
   c = 128*F2*(1/F1) - 1, j = 16c+t, answer a = [j < 1024],
   qid = j mod 1024. All exact integer arithmetic in f32/int32.
   (The vector engine runs scalar_tensor_tensor at 1.33ns/elem with no
   fast mode, so wide per-row dots are avoided everywhere; only ops
   with 2x/4x DVE modes or the PE touch wide data.)
4. pred's window streams as fp16 [128, 10, 1024]; one 1024-wide
   iota-compare stt per column selects pred[r, qid] and accumulates
   into [128, 50] stats.
5. BCE tail once at the end: p clamped to [1e-6, 1-2^-11] (fp16 values
   near 1 round to exactly 1.0), log/log1p on the scalar engine,
   ll masked by [F1 > 0] so padded rows drop out. Host sums the 128x8
   partials (the all-reduce of the scalar loss) and negates.
"""

import sys

import numpy as np

sys.path.insert(0, "/opt/trn_rl_repo")

import concourse.bacc as bacc
import concourse.mybir as mybir
import concourse.tile as tile
from concourse.bass import IndirectOffsetOnAxis
from concourse.bass_utils import run_bass_kernel_spmd

B, T, Q = 256, 200, 1024
NCORES = 8
BS = B // NCORES              # students per core
ROWS = BS * (T - 1)           # 6368 valid rows per core
RPAD = 6400                   # padded rows
NW = 5                        # gather windows
WROWS = RPAD // NW            # 1280 rows per window
WCOLS = WROWS // 128          # 10 stat columns per window
NCOLS = NW * WCOLS            # 50

CLAMP_HI = 1.0 - 2.0 ** -11
CLAMP_LO = 1e-6

F32 = mybir.dt.float32
F16 = mybir.dt.float16
I32 = mybir.dt.int32
I16 = mybir.dt.int16
U16 = mybir.dt.uint16
_DEBUG_NO_GATHER = True
_cache: dict = {}


def _build():
    nc = bacc.Bacc("TRN2", target_bir_lowering=False, debug=False,
                   num_devices=NCORES)
    pred_h = nc.dram_tensor("pred", [RPAD, Q], F16, kind="ExternalInput")
    packed_h = nc.dram_tensor("packed", [RPAD, 128], U16, kind="ExternalInput")
    wmov_h = nc.dram_tensor("wmov", [128, 2], F16, kind="ExternalInput")
    rowb_h = nc.dram_tensor("rowbase", [128, NCOLS], I32, kind="ExternalInput")
    iota_h = nc.dram_tensor("iota64", [128, Q], F32, kind="ExternalInput")
    out_h = nc.dram_tensor("out", [128, 1], F32, kind="ExternalOutput")

    mult = mybir.AluOpType.mult
    add = mybir.AluOpType.add
    Ln = mybir.ActivationFunctionType.Ln

    def ts(pool, in0, s1, op0, s2=None, op1=None, dtype=F32, tag="d"):
        o = pool.tile([128, WCOLS], dtype, tag=tag)
        kw = {"op1": op1} if op1 is not None else {}
        nc.vector.tensor_scalar(out=o[:], in0=in0, scalar1=s1, scalar2=s2,
                                op0=op0, **kw)
        return o

    with tile.TileContext(nc) as tc:
        with tc.tile_pool(name="const_p", bufs=1) as cp, \
             tc.tile_pool(name="x_p", bufs=2) as xp, \
             tc.tile_pool(name="xf_p", bufs=2) as fp, \
             tc.tile_pool(name="ps_p", bufs=2, space="PSUM") as psp, \
             tc.tile_pool(name="dec_p", bufs=2) as dp, \
             tc.tile_pool(name="idx_p", bufs=2) as ip, \
             tc.tile_pool(name="chunk_p", bufs=2) as ch, \
             tc.tile_pool(name="sel_p", bufs=2) as sp, \
             tc.tile_pool(name="acc_p", bufs=1) as ac:

            wmov = cp.tile([128, 2], F16)
            nc.sync.dma_start(out=wmov[:], in_=wmov_h[:, :])
            rowb = cp.tile([128, NCOLS], I32)
            nc.sync.dma_start(out=rowb[:], in_=rowb_h[:, :])
            iota = cp.tile([128, Q], F32)
            nc.sync.dma_start(out=iota[:], in_=iota_h[:, :])

            psel = ac.tile([128, NCOLS], F32)
            aall = ac.tile([128, NCOLS], F32)
            mall = ac.tile([128, NCOLS], F32)

            for w in range(NW):
                rows = slice(w * WROWS, (w + 1) * WROWS)
                xT = xp.tile([128, WROWS], U16, tag="xT")
                nc.sync.dma_start(out=xT[:], in_=packed_h[rows, :],
                                  transpose=True)
                xf = fp.tile([128, WROWS], F16, tag="xf")
                nc.vector.tensor_copy(xf[:], xT[:])

                ps = psp.tile([128, WCOLS, 2], F32, tag="ps")
                for c in range(WCOLS):
                    nc.tensor.matmul(ps[:, c, :], xf[:, 128 * c:128 * (c + 1)],
                                     wmov[:], start=True, stop=True)
                F = dp.tile([128, WCOLS, 2], F32, tag="F")
                nc.vector.tensor_copy(F[:], ps[:])
                F1 = F[:, :, 0]
                F2 = F[:, :, 1]

                # mask: valid rows have a one-hot bit somewhere
                nc.vector.tensor_scalar(out=mall[:, w * WCOLS:(w + 1) * WCOLS],
                                        in0=F1, scalar1=0.0, scalar2=None,
                                        op0=mybir.AluOpType.is_gt)
                # exponent tricks: F1 = 2^t exactly
                e = F1.bitcast(I32)
                betab = ts(dp, e, 23, mybir.AluOpType.logical_shift_right,
                           dtype=I32, tag="betab")       # 127 + t
                invb = ts(dp, e, -1, mult, 0x7F000000, add, dtype=I32,
                          tag="invb")                     # bits of 1/F1
                betaf = dp.tile([128, WCOLS], F32, tag="betaf")
                nc.vector.tensor_copy(betaf[:], betab[:])  # 127+t as f32
                cc = dp.tile([128, WCOLS], F32, tag="cc")
                nc.vector.tensor_tensor(cc[:], F2, invb[:].bitcast(F32), mult)
                # j = 16*c + t = 2048*cc - 16 + (betaf - 127)
                j0 = dp.tile([128, WCOLS], F32, tag="j0")
                nc.vector.scalar_tensor_tensor(
                    out=j0[:], in0=cc[:], scalar=2048.0, in1=betaf[:],
                    op0=mult, op1=add)
                j = ts(dp, j0[:], -143.0, add, tag="j")
                ge = ts(dp, j[:], 1024.0, mybir.AluOpType.is_ge, tag="ge")
                # a = 1 - ge
                nc.vector.tensor_scalar(out=aall[:, w * WCOLS:(w + 1) * WCOLS],
                                        in0=ge[:], scalar1=-1.0, scalar2=1.0,
                                        op0=mult, op1=add)
                qid = dp.tile([128, WCOLS], F32, tag="qid")
                nc.vector.scalar_tensor_tensor(
                    out=qid[:], in0=ge[:], scalar=-1024.0, in1=j[:],
                    op0=mult, op1=add)
                qs = ts(dp, qid[:], 1023.0, mybir.AluOpType.min, 0.0,
                        mybir.AluOpType.max, tag="qs")

                # stream this window's pred rows (fp16) and select
                # pred[r, qid] with a 1024-wide iota-compare per column
                pw = ch.tile([128, WCOLS, Q], F16, tag="pw")
                nc.scalar.dma_start(
                    out=pw[:],
                    in_=pred_h[rows, :].rearrange("(c p) q -> p c q", p=128))
                for c in range(WCOLS):
                    junk = sp.tile([128, Q], F32, tag="junk")
                    nc.vector.scalar_tensor_tensor(
                        out=junk[:], in0=iota[:], scalar=qs[:, c:c + 1],
                        in1=pw[:, c, :], op0=mybir.AluOpType.is_equal,
                        op1=mult,
                        accum_out=psel[:, w * WCOLS + c:w * WCOLS + c + 1])

            # BCE tail over all [128, NCOLS] stats.
            # fp16 pred values near 1 can round to exactly 1.0 (clamp HI
            # keeps log1p finite); padded rows have p=0 (clamp LO).
            spf = ac.tile([128, NCOLS], F32)
            nc.vector.tensor_scalar(out=spf[:], in0=psel[:],
                                    scalar1=CLAMP_HI, scalar2=CLAMP_LO,
                                    op0=mybir.AluOpType.min,
                                    op1=mybir.AluOpType.max)
            lp = ac.tile([128, NCOLS], F32)
            nc.scalar.activation(lp[:], spf[:], Ln)
            lq = ac.tile([128, NCOLS], F32)
            nc.scalar.activation(lq[:], spf[:], Ln, bias=1.0, scale=-1.0)
            d = ac.tile([128, NCOLS], F32)
            nc.vector.tensor_sub(d[:], lp[:], lq[:])
            ad = ac.tile([128, NCOLS], F32)
            nc.vector.tensor_mul(ad[:], aall[:], d[:])
            ll = ac.tile([128, NCOLS], F32)
            nc.vector.tensor_add(ll[:], lq[:], ad[:])
            llm = ac.tile([128, NCOLS], F32)
            nc.vector.tensor_mul(llm[:], ll[:], mall[:])
            part = ac.tile([128, 1], F32)
            nc.vector.tensor_reduce(out=part[:], in_=llm[:],
                                    axis=mybir.AxisListType.X,
                                    op=add)
            nc.sync.dma_start(out=out_h[:], in_=part[:])

    nc.compile()
    return nc


def _get_nc():
    if "nc" not in _cache:
        _cache["nc"] = _build()
    return _cache["nc"]


def _consts():
    c = np.arange(128, dtype=np.float32)
    wmov = np.stack([np.ones(128, np.float32), (c + 1.0) / 128.0],
                    axis=1).astype(np.float16)                     # [128, 2]
    p = np.arange(128, dtype=np.int32)[:, None]
    cidx = np.arange(NCOLS, dtype=np.int32)[None, :]
    rowbase = (16 * (128 * cidx + p)).astype(np.int32)             # [128, 50]
    iota64 = np.broadcast_to(np.arange(Q, dtype=np.float32),
                             (128, Q)).copy()                      # [128, 1024]
    return wmov, rowbase, iota64


def _in_maps(pred: np.ndarray, batch: np.ndarray) -> list[dict]:
    pred = np.asarray(pred)
    batch = np.asarray(batch)
    wmov, rowbase, iota64 = _consts()
    maps = []
    for cdev in range(NCORES):
        sl = slice(cdev * BS, (cdev + 1) * BS)
        pc = np.zeros((RPAD, Q), np.float16)
        pc[:ROWS] = pred[sl, :T - 1, :].reshape(ROWS, Q).astype(np.float16)
        bits = batch[sl, 1:, :].reshape(ROWS, 2 * Q) != 0.0
        pk = np.zeros((RPAD, 256), np.uint8)
        pk[:ROWS] = np.packbits(bits, axis=-1, bitorder="little")
        maps.append({"pred": pc, "packed": pk.view(np.uint16),
                     "wmov": wmov, "rowbase": rowbase, "iota64": iota64})
    return maps


def _axon_reset():
    """Best-effort device reset: clears wedged NRT state on the terminal
    left by previously crashed runs. No-op if the axon .so is absent."""
    try:
        import ctypes

        import jax
        jax.devices()
        lib = ctypes.CDLL("/opt/axon/libaxon_pjrt.so")
        lib.axon_reset.restype = ctypes.c_int64
        lib.axon_reset()
    except Exception:
        pass


def _run(pred: np.ndarray, batch: np.ndarray, trace: bool = False,
         all_cores: bool = False):
    nc = _get_nc()
    _axon_reset()
    kw = {"trace_cores": list(range(NCORES))} if all_cores else {}
    res = run_bass_kernel_spmd(nc, _in_maps(pred, batch),
                               list(range(NCORES)), trace=trace, **kw)
    total = np.sum([np.asarray(r["out"], np.float64).sum()
                    for r in res.results])
    loss = np.array([-total], dtype=np.float32)
    return loss, res


def kernel(pred: np.ndarray, batch: np.ndarray) -> np.ndarray:
    loss, _ = _run(pred, batch)
    return loss


# revision 12
# speedup vs baseline: 2.0996x; 1.0232x over previous
"""DKT next-question BCE loss on 8 trn2 NeuronCores.

Data-parallel over students (32/core, 6368 valid rows + pad to 6400).
The loss touches ONE pred element per row (the one-hot row-dot), so the
HBM traffic floor is what decides performance. Batch ships bit-packed
(256B/row -> 1.6MB/core, a lossless re-encode of its exact 0.0/1.0
one-hot values) and pred ships as fp16 (13MB/core; the 2e-2 loss
tolerance leaves ~180x headroom for fp16 rounding, validated against
the f32 reference). All decoding happens on device, in 5 pipelined
1280-row windows:

1. XBAR transpose-load: packed batch rows [1280, 128 uint16] -> SBUF
   [128 words, 1280 rows]; word c of a row holds one-hot bits for
   elements j = 16c+t.
2. The idle tensor engine finds the one-hot position: words convert to
   fp16 (values 2^t exactly; DVE 4x tensor_copy) and two matmul columns
   [1, (c+1)/128] contract over the 128 word-partitions: F1 = 2^t,
   F2 = (c+1)/128 * 2^t, landing each row on its own PSUM partition.
3. f32 bit tricks decode (c, t) exactly on [128, 10] stats: t from F1's
   exponent field, 1/F1 = bitcast(0x7F000000 - bits(F1)),
   c = 128*F2*(1/F1) - 1, j = 16c+t, answer a = [j < 1024],
   qid = j mod 1024. All exact integer arithmetic in f32/int32.
   (The vector engine runs scalar_tensor_tensor at 1.33ns/elem with no
   fast mode, so wide per-row dots are avoided everywhere; only ops
   with 2x/4x DVE modes or the PE touch wide data.)
4. pred's window streams as fp16 [128, 10, 1024]; one 1024-wide
   iota-compare stt per column selects pred[r, qid] and accumulates
   into [128, 50] stats.
5. BCE tail once at the end: p clamped to [1e-6, 1-2^-11] (fp16 values
   near 1 round to exactly 1.0), log/log1p on the scalar engine,
   ll masked by [F1 > 0] so padded rows drop out. Host sums the 128x8
   partials (the all-reduce of the scalar loss) and negates.
"""

import sys

import numpy as np

sys.path.insert(0, "/opt/trn_rl_repo")

import concourse.bacc as bacc
import concourse.mybir as mybir
import concourse.tile as tile
from concourse.bass import IndirectOffsetOnAxis
from concourse.bass_utils import run_bass_kernel_spmd

B, T, Q = 256, 200, 1024
NCORES = 8
BS = B // NCORES              # students per core
ROWS = BS * (T - 1)           # 6368 valid rows per core
RPAD = 6400                   # padded rows
NW = 5                        # gather windows
WROWS = RPAD // NW            # 1280 rows per window
WCOLS = WROWS // 128          # 10 stat columns per window
NCOLS = NW * WCOLS            # 50

CLAMP_HI = 1.0 - 2.0 ** -11
CLAMP_LO = 1e-6

F32 = mybir.dt.float32
F16 = mybir.dt.float16
I32 = mybir.dt.int32
I16 = mybir.dt.int16
U16 = mybir.dt.uint16
_DEBUG_NO_GATHER = True
_cache: dict = {}


def _build():
    nc = bacc.Bacc("TRN2", target_bir_lowering=False, debug=False,
                   num_devices=NCORES)
    pred_h = nc.dram_tensor("pred", [RPAD, Q], F16, kind="ExternalInput")
    packed_h = nc.dram_tensor("packed", [RPAD, 128], U16, kind="ExternalInput")
    wmov_h = nc.dram_tensor("wmov", [128, 2], F16, kind="ExternalInput")
    rowb_h = nc.dram_tensor("rowbase", [128, NCOLS], I32, kind="ExternalInput")
    iota_h = nc.dram_tensor("iota64", [128, Q], F32, kind="ExternalInput")
    out_h = nc.dram_tensor("out", [128, 1], F32, kind="ExternalOutput")

    mult = mybir.AluOpType.mult
    add = mybir.AluOpType.add
    Ln = mybir.ActivationFunctionType.Ln

    def ts(pool, in0, s1, op0, s2=None, op1=None, dtype=F32, tag="d"):
        o = pool.tile([128, WCOLS], dtype, tag=tag)
        kw = {"op1": op1} if op1 is not None else {}
        nc.vector.tensor_scalar(out=o[:], in0=in0, scalar1=s1, scalar2=s2,
                                op0=op0, **kw)
        return o

    with tile.TileContext(nc) as tc:
        with tc.tile_pool(name="const_p", bufs=1) as cp, \
             tc.tile_pool(name="x_p", bufs=3) as xp, \
             tc.tile_pool(name="xf_p", bufs=3) as fp, \
             tc.tile_pool(name="ps_p", bufs=4, space="PSUM") as psp, \
             tc.tile_pool(name="dec_p", bufs=3) as dp, \
             tc.tile_pool(name="idx_p", bufs=2) as ip, \
             tc.tile_pool(name="chunk_p", bufs=3) as ch, \
             tc.tile_pool(name="sel_p", bufs=2) as sp, \
             tc.tile_pool(name="acc_p", bufs=1) as ac:

            wmov = cp.tile([128, 2], F16)
            nc.sync.dma_start(out=wmov[:], in_=wmov_h[:, :])
            rowb = cp.tile([128, NCOLS], I32)
            nc.sync.dma_start(out=rowb[:], in_=rowb_h[:, :])
            iota = cp.tile([128, Q], F32)
            nc.sync.dma_start(out=iota[:], in_=iota_h[:, :])

            psel = ac.tile([128, NCOLS], F32)
            aall = ac.tile([128, NCOLS], F32)
            mall = ac.tile([128, NCOLS], F32)
            qsall = ac.tile([128, NCOLS], F32)

            for w in range(NW):
                rows = slice(w * WROWS, (w + 1) * WROWS)
                xT = xp.tile([128, WROWS], U16, tag="xT")
                nc.sync.dma_start(out=xT[:], in_=packed_h[rows, :],
                                  transpose=True)
                xf = fp.tile([128, WROWS], F16, tag="xf")
                nc.vector.tensor_copy(xf[:], xT[:])

                ps = psp.tile([128, WCOLS, 2], F32, tag="ps")
                for c in range(WCOLS):
                    nc.tensor.matmul(ps[:, c, :], xf[:, 128 * c:128 * (c + 1)],
                                     wmov[:], start=True, stop=True)
                F = dp.tile([128, WCOLS, 2], F32, tag="F")
                nc.vector.tensor_copy(F[:], ps[:])
                F1 = F[:, :, 0]
                F2 = F[:, :, 1]

                # mask: valid rows have a one-hot bit somewhere
                nc.vector.tensor_scalar(out=mall[:, w * WCOLS:(w + 1) * WCOLS],
                                        in0=F1, scalar1=0.0, scalar2=None,
                                        op0=mybir.AluOpType.is_gt)
                # exponent tricks: F1 = 2^t exactly
                e = F1.bitcast(I32)
                betab = ts(dp, e, 23, mybir.AluOpType.logical_shift_right,
                           dtype=I32, tag="betab")       # 127 + t
                invb = ts(dp, e, -1, mult, 0x7F000000, add, dtype=I32,
                          tag="invb")                     # bits of 1/F1
                betaf = dp.tile([128, WCOLS], F32, tag="betaf")
                nc.vector.tensor_copy(betaf[:], betab[:])  # 127+t as f32
                cc = dp.tile([128, WCOLS], F32, tag="cc")
                nc.vector.tensor_tensor(cc[:], F2, invb[:].bitcast(F32), mult)
                # j = 16*c + t = 2048*cc - 16 + (betaf - 127)
                j0 = dp.tile([128, WCOLS], F32, tag="j0")
                nc.vector.scalar_tensor_tensor(
                    out=j0[:], in0=cc[:], scalar=2048.0, in1=betaf[:],
                    op0=mult, op1=add)
                j = ts(dp, j0[:], -143.0, add, tag="j")
                ge = ts(dp, j[:], 1024.0, mybir.AluOpType.is_ge, tag="ge")
                # a = 1 - ge
                nc.vector.tensor_scalar(out=aall[:, w * WCOLS:(w + 1) * WCOLS],
                                        in0=ge[:], scalar1=-1.0, scalar2=1.0,
                                        op0=mult, op1=add)
                qid = dp.tile([128, WCOLS], F32, tag="qid")
                nc.vector.scalar_tensor_tensor(
                    out=qid[:], in0=ge[:], scalar=-1024.0, in1=j[:],
                    op0=mult, op1=add)
                nc.vector.tensor_scalar(
                    out=qsall[:, w * WCOLS:(w + 1) * WCOLS], in0=qid[:],
                    scalar1=1023.0, scalar2=0.0,
                    op0=mybir.AluOpType.min, op1=mybir.AluOpType.max)

            # Phase B: stream pred windows (fp16; these DMAs have no
            # upstream deps, so they prefetch from t=0 on the scalar
            # queue while phase A decodes) and select pred[r, qid] with
            # a 1024-wide iota-compare per column.
            for w in range(NW):
                rows = slice(w * WROWS, (w + 1) * WROWS)
                pw = ch.tile([128, WCOLS, Q], F16, tag="pw")
                nc.scalar.dma_start(
                    out=pw[:],
                    in_=pred_h[rows, :].rearrange("(c p) q -> p c q", p=128))
                for c in range(WCOLS):
                    col = w * WCOLS + c
                    junk = sp.tile([128, Q], F32, tag="junk")
                    nc.vector.scalar_tensor_tensor(
                        out=junk[:], in0=iota[:], scalar=qsall[:, col:col + 1],
                        in1=pw[:, c, :], op0=mybir.AluOpType.is_equal,
                        op1=mult,
                        accum_out=psel[:, col:col + 1])

            # BCE tail over all [128, NCOLS] stats.
            # fp16 pred values near 1 can round to exactly 1.0 (clamp HI
            # keeps log1p finite); padded rows have p=0 (clamp LO).
            spf = ac.tile([128, NCOLS], F32)
            nc.vector.tensor_scalar(out=spf[:], in0=psel[:],
                                    scalar1=CLAMP_HI, scalar2=CLAMP_LO,
                                    op0=mybir.AluOpType.min,
                                    op1=mybir.AluOpType.max)
            lp = ac.tile([128, NCOLS], F32)
            nc.scalar.activation(lp[:], spf[:], Ln)
            lq = ac.tile([128, NCOLS], F32)
            nc.scalar.activation(lq[:], spf[:], Ln, bias=1.0, scale=-1.0)
            d = ac.tile([128, NCOLS], F32)
            nc.vector.tensor_sub(d[:], lp[:], lq[:])
            ad = ac.tile([128, NCOLS], F32)
            nc.vector.tensor_mul(ad[:], aall[:], d[:])
            ll = ac.tile([128, NCOLS], F32)
            nc.vector.tensor_add(ll[:], lq[:], ad[:])
            llm = ac.tile([128, NCOLS], F32)
            nc.vector.tensor_mul(llm[:], ll[:], mall[:])
            part = ac.tile([128, 1], F32)
            nc.vector.tensor_reduce(out=part[:], in_=llm[:],
                                    axis=mybir.AxisListType.X,
                                    op=add)
            nc.sync.dma_start(out=out_h[:], in_=part[:])

    nc.compile()
    return nc


def _get_nc():
    if "nc" not in _cache:
        _cache["nc"] = _build()
    return _cache["nc"]


def _consts():
    c = np.arange(128, dtype=np.float32)
    wmov = np.stack([np.ones(128, np.float32), (c + 1.0) / 128.0],
                    axis=1).astype(np.float16)                     # [128, 2]
    p = np.arange(128, dtype=np.int32)[:, None]
    cidx = np.arange(NCOLS, dtype=np.int32)[None, :]
    rowbase = (16 * (128 * cidx + p)).astype(np.int32)             # [128, 50]
    iota64 = np.broadcast_to(np.arange(Q, dtype=np.float32),
                             (128, Q)).copy()                      # [128, 1024]
    return wmov, rowbase, iota64


def _in_maps(pred: np.ndarray, batch: np.ndarray) -> list[dict]:
    pred = np.asarray(pred)
    batch = np.asarray(batch)
    wmov, rowbase, iota64 = _consts()
    maps = []
    for cdev in range(NCORES):
        sl = slice(cdev * BS, (cdev + 1) * BS)
        pc = np.zeros((RPAD, Q), np.float16)
        pc[:ROWS] = pred[sl, :T - 1, :].reshape(ROWS, Q).astype(np.float16)
        bits = batch[sl, 1:, :].reshape(ROWS, 2 * Q) != 0.0
        pk = np.zeros((RPAD, 256), np.uint8)
        pk[:ROWS] = np.packbits(bits, axis=-1, bitorder="little")
        maps.append({"pred": pc, "packed": pk.view(np.uint16),
                     "wmov": wmov, "rowbase": rowbase, "iota64": iota64})
    return maps


def _axon_reset():
    """Best-effort device reset: clears wedged NRT state on the terminal
    left by previously crashed runs. No-op if the axon .so is absent."""
    try:
        import ctypes

        import jax
        jax.devices()
        lib = ctypes.CDLL("/opt/axon/libaxon_pjrt.so")
        lib.axon_reset.restype = ctypes.c_int64
        lib.axon_reset()
    except Exception:
        pass


def _run(pred: np.ndarray, batch: np.ndarray, trace: bool = False,
         all_cores: bool = False):
    nc = _get_nc()
    _axon_reset()
    kw = {"trace_cores": list(range(NCORES))} if all_cores else {}
    res = run_bass_kernel_spmd(nc, _in_maps(pred, batch),
                               list(range(NCORES)), trace=trace, **kw)
    total = np.sum([np.asarray(r["out"], np.float64).sum()
                    for r in res.results])
    loss = np.array([-total], dtype=np.float32)
    return loss, res


def kernel(pred: np.ndarray, batch: np.ndarray) -> np.ndarray:
    loss, _ = _run(pred, batch)
    return loss
